# revision 1
# baseline (speedup 1.0000x reference)
"""Depthwise 31x31 conv (32,384,56,56) on 8 TRN2 NeuronCores.

Strategy: channel-shard 384 -> 48 per core (depthwise is per-channel
independent, no communication). Per channel, the 2D conv is computed on
the TensorEngine as 32 PSUM-accumulated matmuls:
  stationary lhsT [K=112, M=112]: K = (2 w-shift copies x 56 h_in),
  M = (w-parity x 56 h_out), holding a Toeplitz-over-h arrangement of the
  31x31 taps (host-precomputed table; kw is covered by the d-step index,
  the w-shift copy rc, and the output w-parity wr: kw = 2d + rc - wr).
  moving rhs [112, 448]: 16 images x 28 w-blocks read strided from a
  host-prepadded x layout. Output returns in PSUM-native layout and is
  reassembled on the host.
"""

import sys

sys.path.insert(0, "/opt/trn_rl_repo")

import numpy as np
import concourse.bacc as bacc
import concourse.mybir as mybir
import concourse.tile as tile
from concourse.bass_utils import run_bass_kernel_spmd

H = W = 56
KK = 31
PAD = 15
ND = 16
WS = 86
KP = 112
MP = 112
N_CORES = 8
C_TOTAL = 384
IMGS = 32
C_PER = C_TOTAL // N_CORES

_DT_IN = mybir.dt.float16
_MPAD = 128

_nc_cache = {}


def _host_prepare(x, weight, io_dtype, mpad=MP):
    C = x.shape[1]
    imgs = x.shape[0]
    xpre = np.zeros((C, 2, H, imgs, WS), dtype=io_dtype)
    xc = np.ascontiguousarray(x.transpose(1, 2, 0, 3)).astype(io_dtype)
    for rc in range(2):
        xpre[:, rc, :, :, 15 - rc:15 - rc + W] = xc
    xpre = xpre.reshape(C, KP, imgs, WS)

    w = weight.reshape(C, KK, KK).astype(np.float32)
    wtab = np.zeros((C, 2, H, ND, 2, H), dtype=np.float32)
    hi = np.arange(H)[:, None]
    ho = np.arange(H)[None, :]
    kh = hi - ho + PAD
    khv = (kh >= 0) & (kh < KK)
    khc = np.clip(kh, 0, KK - 1)
    for d in range(ND):
        for rc in range(2):
            for wr in range(2):
                kw = 2 * d + rc - wr
                if not (0 <= kw < KK):
                    continue
                wtab[:, rc, :, d, wr, :] = w[:, khc, kw] * khv[None, :, :]
    wtab = wtab.reshape(C, KP, ND, MP)
    if mpad > MP:
        wtab = np.concatenate(
            [wtab, np.zeros((C, KP, ND, mpad - MP), wtab.dtype)], axis=3)
    return xpre, wtab.astype(io_dtype)


def _build_nc(C, imgs, n_cores, dt_in, repeat=1, mpad=MP, loop_repeat=1,
              d_outer=False):
    f32 = mybir.dt.float32
    is_f32r = dt_in == mybir.dt.float32r
    dt_store = f32 if is_f32r else dt_in

    nc = bacc.Bacc("TRN2", target_bir_lowering=False, debug=False,
                   num_devices=n_cores)
    xp = nc.dram_tensor("xp", [C, KP, imgs, WS], dt_store, kind="ExternalInput")
    wt = nc.dram_tensor("wt", [C, KP, ND, mpad], dt_store, kind="ExternalInput")
    yt = nc.dram_tensor("yt", [C, MP, imgs, W // 2], f32, kind="ExternalOutput")
    xp_ap, wt_ap, yt_ap = xp.ap(), wt.ap(), yt.ap()

    half_sz = 16
    halves = (imgs + half_sz - 1) // half_sz

    with tile.TileContext(nc) as tc:
        with (
            tc.tile_pool(name="xpool", bufs=3) as xpool,
            tc.tile_pool(name="wpool", bufs=3) as wpool,
            tc.tile_pool(name="ypool", bufs=3) as ypool,
            tc.tile_pool(name="psum", bufs=4, space="PSUM") as psum,
        ):
            def body(_iv=None):
                for c in [ci for _ in range(repeat) for ci in range(C)]:
                    xt = xpool.tile([KP, imgs, WS], dt_store)
                    nc.sync.dma_start(xt[:], xp_ap[c])
                    wtt = wpool.tile([KP, ND, mpad], dt_store)
                    nc.sync.dma_start(wtt[:], wt_ap[c])
                    ytile = ypool.tile([MP, imgs, W // 2], f32)
                    bounds = [(half_sz * hf, min(imgs, half_sz * hf + half_sz))
                              for hf in range(halves)]

                    def mm(ps, d, i0, i1):
                        lhsT = wtt[:, d, :]
                        rhs = xt[:, i0:i1, 2 * d: 2 * d + W: 2]
                        if is_f32r:
                            lhsT = lhsT.bitcast(dt_in)
                            rhs = rhs.bitcast(dt_in)
                        nc.tensor.matmul(ps[:], lhsT, rhs,
                                         start=(d == 0), stop=(d == ND - 1))

                    if d_outer:
                        tiles = [psum.tile([mpad, i1 - i0, W // 2], f32,
                                           name=f"ps{hf}", tag=f"ps{hf}")
                                 for hf, (i0, i1) in enumerate(bounds)]
                        for d in range(ND):
                            for ps, (i0, i1) in zip(tiles, bounds):
                                mm(ps, d, i0, i1)
                        for ps, (i0, i1) in zip(tiles, bounds):
                            nc.vector.tensor_copy(ytile[:, i0:i1, :],
                                                  ps[:MP, :, :])
                    else:
                        for (i0, i1) in bounds:
                            ps = psum.tile([mpad, i1 - i0, W // 2], f32)
                            for d in range(ND):
                                mm(ps, d, i0, i1)
                            nc.vector.tensor_copy(ytile[:, i0:i1, :],
                                                  ps[:MP, :, :])
                    nc.sync.dma_start(yt_ap[c], ytile[:])

            if loop_repeat > 1:
                with tc.For_i(0, loop_repeat, 1):
                    body()
            else:
                body()
    nc.compile()
    return nc


def _get_nc():
    key = (C_PER, IMGS, N_CORES, _DT_IN)
    if key not in _nc_cache:
        _nc_cache[key] = _build_nc(*key, mpad=_MPAD)
    return _nc_cache[key]


def kernel(x, weight, bias_term):
    x = np.asarray(x, dtype=np.float32)
    weight = np.asarray(weight, dtype=np.float32)
    bias_term = np.asarray(bias_term, dtype=np.float32)

    nc = _get_nc()
    np_dt = mybir.dt.np(mybir.dt.float32 if _DT_IN == mybir.dt.float32r
                        else _DT_IN)
    xpre, wtab = _host_prepare(x, weight, np_dt, mpad=_MPAD)

    in_maps = []
    for k in range(N_CORES):
        c0 = k * C_PER
        in_maps.append({
            "xp": np.ascontiguousarray(xpre[c0:c0 + C_PER]),
            "wt": np.ascontiguousarray(wtab[c0:c0 + C_PER]),
        })
    res = run_bass_kernel_spmd(nc, in_maps, list(range(N_CORES)))

    y = np.empty((IMGS, C_TOTAL, H, W), np.float32)
    for k in range(N_CORES):
        c0 = k * C_PER
        yt = res.results[k]["yt"]                      # [C_PER, 112, IMGS, 28]
        yk = yt.reshape(C_PER, 2, H, IMGS, W // 2)     # [c, wr, h, img, bw]
        # -> [img, c, h, bw, wr] -> [img, c, h, w]
        y[:, c0:c0 + C_PER] = yk.transpose(3, 0, 2, 4, 1).reshape(
            IMGS, C_PER, H, W)
    y += bias_term[None, :, None, None]
    return y



# revision 4
# speedup vs baseline: 1.0146x; 1.0146x over previous
"""Depthwise 31x31 conv (32,384,56,56) on 8 TRN2 NeuronCores.

Strategy: channel-shard 384 -> 48 per core (depthwise is per-channel
independent, no communication). Per channel, the 2D conv is computed on
the TensorEngine as 32 PSUM-accumulated matmuls:
  stationary lhsT [K=112, M=112]: K = (2 w-shift copies x 56 h_in),
  M = (w-parity x 56 h_out), holding a Toeplitz-over-h arrangement of the
  31x31 taps (host-precomputed table; kw is covered by the d-step index,
  the w-shift copy rc, and the output w-parity wr: kw = 2d + rc - wr).
  moving rhs [112, 448]: 16 images x 28 w-blocks read strided from a
  host-prepadded x layout. Output returns in PSUM-native layout and is
  reassembled on the host.
"""

import sys

sys.path.insert(0, "/opt/trn_rl_repo")

import numpy as np
import concourse.bacc as bacc
import concourse.mybir as mybir
import concourse.tile as tile
from concourse.bass_utils import run_bass_kernel_spmd

H = W = 56
KK = 31
PAD = 15
ND = 16
WS = 86
KP = 112
MP = 112
N_CORES = 8
C_TOTAL = 384
IMGS = 32
C_PER = C_TOTAL // N_CORES

_DT_IN = mybir.dt.float16
_MPAD = 128

_nc_cache = {}


def _host_prepare(x, weight, io_dtype, mpad=MP):
    C = x.shape[1]
    imgs = x.shape[0]
    xpre = np.zeros((C, 2, H, imgs, WS), dtype=io_dtype)
    xc = x.transpose(1, 2, 0, 3).astype(io_dtype)
    for rc in range(2):
        xpre[:, rc, :, :, 15 - rc:15 - rc + W] = xc
    xpre = xpre.reshape(C, KP, imgs, WS)

    w = weight.reshape(C, KK, KK).astype(np.float32)
    hi = np.arange(H)[:, None]
    ho = np.arange(H)[None, :]
    kh = hi - ho + PAD
    khv = (kh >= 0) & (kh < KK)
    khc = np.clip(kh, 0, KK - 1)
    rc_ = np.arange(2)[:, None, None]
    d_ = np.arange(ND)[None, :, None]
    wr_ = np.arange(2)[None, None, :]
    kw = 2 * d_ + rc_ - wr_                      # [2, ND, 2]
    kwv = (kw >= 0) & (kw < KK)
    kwc = np.clip(kw, 0, KK - 1)
    # wtab[c, rc, hi, d, wr, ho] = w[c, kh(hi,ho), kw(rc,d,wr)] * valid
    wtab = w[:, khc[None, :, None, None, :], kwc[:, None, :, :, None]]
    wtab *= khv[None, :, None, None, :] & kwv[:, None, :, :, None]
    wtab = wtab.reshape(C, KP, ND, MP)
    if mpad > MP:
        wtab = np.concatenate(
            [wtab, np.zeros((C, KP, ND, mpad - MP), wtab.dtype)], axis=3)
    return xpre, wtab.astype(io_dtype)


def _build_nc(C, imgs, n_cores, dt_in, repeat=1, mpad=MP, loop_repeat=1,
              d_outer=False):
    f32 = mybir.dt.float32
    is_f32r = dt_in == mybir.dt.float32r
    dt_store = f32 if is_f32r else dt_in

    dt_out = mybir.dt.float16 if dt_in == mybir.dt.float16 else f32

    nc = bacc.Bacc("TRN2", target_bir_lowering=False, debug=False,
                   num_devices=n_cores)
    xp = nc.dram_tensor("xp", [C, KP, imgs, WS], dt_store, kind="ExternalInput")
    wt = nc.dram_tensor("wt", [C, KP, ND, mpad], dt_store, kind="ExternalInput")
    yt = nc.dram_tensor("yt", [C, MP, imgs, W // 2], dt_out,
                        kind="ExternalOutput")
    xp_ap, wt_ap, yt_ap = xp.ap(), wt.ap(), yt.ap()

    half_sz = 16
    halves = (imgs + half_sz - 1) // half_sz

    with tile.TileContext(nc) as tc:
        with (
            tc.tile_pool(name="xpool", bufs=3) as xpool,
            tc.tile_pool(name="wpool", bufs=3) as wpool,
            tc.tile_pool(name="ypool", bufs=3) as ypool,
            tc.tile_pool(name="psum", bufs=4, space="PSUM") as psum,
        ):
            def body(_iv=None):
                for c in [ci for _ in range(repeat) for ci in range(C)]:
                    xt = xpool.tile([KP, imgs, WS], dt_store)
                    nc.sync.dma_start(xt[:], xp_ap[c])
                    wtt = wpool.tile([KP, ND, mpad], dt_store)
                    nc.sync.dma_start(wtt[:], wt_ap[c])
                    ytile = ypool.tile([MP, imgs, W // 2], dt_out)
                    bounds = [(half_sz * hf, min(imgs, half_sz * hf + half_sz))
                              for hf in range(halves)]

                    def mm(ps, d, i0, i1):
                        lhsT = wtt[:, d, :]
                        rhs = xt[:, i0:i1, 2 * d: 2 * d + W: 2]
                        if is_f32r:
                            lhsT = lhsT.bitcast(dt_in)
                            rhs = rhs.bitcast(dt_in)
                        nc.tensor.matmul(ps[:], lhsT, rhs,
                                         start=(d == 0), stop=(d == ND - 1))

                    if d_outer:
                        tiles = [psum.tile([mpad, i1 - i0, W // 2], f32,
                                           name=f"ps{hf}", tag=f"ps{hf}")
                                 for hf, (i0, i1) in enumerate(bounds)]
                        for d in range(ND):
                            for ps, (i0, i1) in zip(tiles, bounds):
                                mm(ps, d, i0, i1)
                        for ps, (i0, i1) in zip(tiles, bounds):
                            nc.vector.tensor_copy(ytile[:, i0:i1, :],
                                                  ps[:MP, :, :])
                    else:
                        for (i0, i1) in bounds:
                            ps = psum.tile([mpad, i1 - i0, W // 2], f32)
                            for d in range(ND):
                                mm(ps, d, i0, i1)
                            nc.vector.tensor_copy(ytile[:, i0:i1, :],
                                                  ps[:MP, :, :])
                    nc.sync.dma_start(yt_ap[c], ytile[:])

            if loop_repeat > 1:
                with tc.For_i(0, loop_repeat, 1):
                    body()
            else:
                body()
    nc.compile()
    return nc


def _get_nc():
    key = (C_PER, IMGS, N_CORES, _DT_IN)
    if key not in _nc_cache:
        _nc_cache[key] = _build_nc(*key, mpad=_MPAD)
    return _nc_cache[key]


def kernel(x, weight, bias_term):
    x = np.asarray(x, dtype=np.float32)
    weight = np.asarray(weight, dtype=np.float32)
    bias_term = np.asarray(bias_term, dtype=np.float32)

    nc = _get_nc()
    np_dt = mybir.dt.np(mybir.dt.float32 if _DT_IN == mybir.dt.float32r
                        else _DT_IN)
    xpre, wtab = _host_prepare(x, weight, np_dt, mpad=_MPAD)

    in_maps = []
    for k in range(N_CORES):
        c0 = k * C_PER
        in_maps.append({
            "xp": np.ascontiguousarray(xpre[c0:c0 + C_PER]),
            "wt": np.ascontiguousarray(wtab[c0:c0 + C_PER]),
        })
    res = run_bass_kernel_spmd(nc, in_maps, list(range(N_CORES)))

    y = np.empty((IMGS, C_TOTAL, H, W), np.float32)
    for k in range(N_CORES):
        c0 = k * C_PER
        yt = res.results[k]["yt"]                      # [C_PER, 112, IMGS, 28]
        yk = yt.reshape(C_PER, 2, H, IMGS, W // 2)     # [c, wr, h, img, bw]
        # -> [img, c, h, bw, wr] -> [img, c, h, w]
        y[:, c0:c0 + C_PER] = yk.transpose(3, 0, 2, 4, 1).reshape(
            IMGS, C_PER, H, W)
    y += bias_term[None, :, None, None]
    return y



# revision 6
# speedup vs baseline: 1.1966x; 1.1794x over previous
"""Depthwise 31x31 conv (32,384,56,56) on 8 TRN2 NeuronCores.

Strategy: channel-shard 384 -> 48 per core (depthwise is per-channel
independent, no communication). Per channel, the 2D conv is computed on
the TensorEngine as 32 PSUM-accumulated matmuls:
  stationary lhsT [K=112, M=112]: K = (2 w-shift copies x 56 h_in),
  M = (w-parity x 56 h_out), holding a Toeplitz-over-h arrangement of the
  31x31 taps (host-precomputed table; kw is covered by the d-step index,
  the w-shift copy rc, and the output w-parity wr: kw = 2d + rc - wr).
  moving rhs [112, 448]: 16 images x 28 w-blocks read strided from a
  host-prepadded x layout. Output returns in PSUM-native layout as fp16
  (halves output HBM traffic; rel-err budget is 2e-2, measured ~5e-4)
  and is reassembled on the host in fp32.
"""

import sys

sys.path.insert(0, "/opt/trn_rl_repo")

import numpy as np
import concourse.bacc as bacc
import concourse.mybir as mybir
import concourse.tile as tile
from concourse.bass_utils import run_bass_kernel_spmd

H = W = 56
KK = 31
PAD = 15
ND = 16
WS = 86
KP = 112
MP = 112
N_CORES = 8
C_TOTAL = 384
IMGS = 32
C_PER = C_TOTAL // N_CORES

_DT_IN = mybir.dt.float16
_MPAD = 128

_nc_cache = {}


def _host_prepare(x, weight, io_dtype, mpad=None):
    if mpad is None:
        mpad = _MPAD  # must match the dram tensor shape _get_nc() compiled
    C = x.shape[1]
    imgs = x.shape[0]
    xpre = np.zeros((C, 2, H, imgs, WS), dtype=io_dtype)
    xc = x.transpose(1, 2, 0, 3).astype(io_dtype)
    for rc in range(2):
        xpre[:, rc, :, :, 15 - rc:15 - rc + W] = xc
    xpre = xpre.reshape(C, KP, imgs, WS)

    w = weight.reshape(C, KK, KK).astype(np.float32)
    hi = np.arange(H)[:, None]
    ho = np.arange(H)[None, :]
    kh = hi - ho + PAD
    khv = (kh >= 0) & (kh < KK)
    khc = np.clip(kh, 0, KK - 1)
    rc_ = np.arange(2)[:, None, None]
    d_ = np.arange(ND)[None, :, None]
    wr_ = np.arange(2)[None, None, :]
    kw = 2 * d_ + rc_ - wr_                      # [2, ND, 2]
    kwv = (kw >= 0) & (kw < KK)
    kwc = np.clip(kw, 0, KK - 1)
    # wtab[c, rc, hi, d, wr, ho] = w[c, kh(hi,ho), kw(rc,d,wr)] * valid
    wtab = w[:, khc[None, :, None, None, :], kwc[:, None, :, :, None]]
    wtab *= khv[None, :, None, None, :] & kwv[:, None, :, :, None]
    wtab = wtab.reshape(C, KP, ND, MP)
    if mpad > MP:
        wtab = np.concatenate(
            [wtab, np.zeros((C, KP, ND, mpad - MP), wtab.dtype)], axis=3)
    return xpre, wtab.astype(io_dtype)


def _build_nc(C, imgs, n_cores, dt_in, repeat=1, mpad=MP, loop_repeat=1,
              d_outer=False):
    f32 = mybir.dt.float32
    is_f32r = dt_in == mybir.dt.float32r
    dt_store = f32 if is_f32r else dt_in

    dt_out = mybir.dt.float16 if dt_in == mybir.dt.float16 else f32

    nc = bacc.Bacc("TRN2", target_bir_lowering=False, debug=False,
                   num_devices=n_cores)
    xp = nc.dram_tensor("xp", [C, KP, imgs, WS], dt_store, kind="ExternalInput")
    wt = nc.dram_tensor("wt", [C, KP, ND, mpad], dt_store, kind="ExternalInput")
    yt = nc.dram_tensor("yt", [C, MP, imgs, W // 2], dt_out,
                        kind="ExternalOutput")
    xp_ap, wt_ap, yt_ap = xp.ap(), wt.ap(), yt.ap()

    half_sz = 16
    halves = (imgs + half_sz - 1) // half_sz

    with tile.TileContext(nc) as tc:
        with (
            tc.tile_pool(name="xpool", bufs=3) as xpool,
            tc.tile_pool(name="wpool", bufs=3) as wpool,
            tc.tile_pool(name="ypool", bufs=3) as ypool,
            tc.tile_pool(name="psum", bufs=4, space="PSUM") as psum,
        ):
            def body(_iv=None):
                for c in [ci for _ in range(repeat) for ci in range(C)]:
                    xt = xpool.tile([KP, imgs, WS], dt_store)
                    nc.sync.dma_start(xt[:], xp_ap[c])
                    wtt = wpool.tile([KP, ND, mpad], dt_store)
                    nc.sync.dma_start(wtt[:], wt_ap[c])
                    ytile = ypool.tile([MP, imgs, W // 2], dt_out)
                    bounds = [(half_sz * hf, min(imgs, half_sz * hf + half_sz))
                              for hf in range(halves)]

                    def mm(ps, d, i0, i1):
                        lhsT = wtt[:, d, :]
                        rhs = xt[:, i0:i1, 2 * d: 2 * d + W: 2]
                        if is_f32r:
                            lhsT = lhsT.bitcast(dt_in)
                            rhs = rhs.bitcast(dt_in)
                        nc.tensor.matmul(ps[:], lhsT, rhs,
                                         start=(d == 0), stop=(d == ND - 1))

                    if d_outer:
                        tiles = [psum.tile([mpad, i1 - i0, W // 2], f32,
                                           name=f"ps{hf}", tag=f"ps{hf}")
                                 for hf, (i0, i1) in enumerate(bounds)]
                        for d in range(ND):
                            for ps, (i0, i1) in zip(tiles, bounds):
                                mm(ps, d, i0, i1)
                        for ps, (i0, i1) in zip(tiles, bounds):
                            nc.vector.tensor_copy(ytile[:, i0:i1, :],
                                                  ps[:MP, :, :])
                    else:
                        for (i0, i1) in bounds:
                            ps = psum.tile([mpad, i1 - i0, W // 2], f32)
                            for d in range(ND):
                                mm(ps, d, i0, i1)
                            nc.vector.tensor_copy(ytile[:, i0:i1, :],
                                                  ps[:MP, :, :])
                    nc.sync.dma_start(yt_ap[c], ytile[:])

            if loop_repeat > 1:
                with tc.For_i(0, loop_repeat, 1):
                    body()
            else:
                body()
    nc.compile()
    return nc


def _get_nc():
    key = (C_PER, IMGS, N_CORES, _DT_IN)
    if key not in _nc_cache:
        _nc_cache[key] = _build_nc(*key, mpad=_MPAD)
    return _nc_cache[key]


def kernel(x, weight, bias_term):
    x = np.asarray(x, dtype=np.float32)
    weight = np.asarray(weight, dtype=np.float32)
    bias_term = np.asarray(bias_term, dtype=np.float32)

    nc = _get_nc()
    np_dt = mybir.dt.np(mybir.dt.float32 if _DT_IN == mybir.dt.float32r
                        else _DT_IN)
    xpre, wtab = _host_prepare(x, weight, np_dt, mpad=_MPAD)

    in_maps = []
    for k in range(N_CORES):
        c0 = k * C_PER
        in_maps.append({
            "xp": np.ascontiguousarray(xpre[c0:c0 + C_PER]),
            "wt": np.ascontiguousarray(wtab[c0:c0 + C_PER]),
        })
    res = run_bass_kernel_spmd(nc, in_maps, list(range(N_CORES)))

    y = np.empty((IMGS, C_TOTAL, H, W), np.float32)
    for k in range(N_CORES):
        c0 = k * C_PER
        yt = res.results[k]["yt"]                      # [C_PER, 112, IMGS, 28]
        yk = yt.reshape(C_PER, 2, H, IMGS, W // 2)     # [c, wr, h, img, bw]
        # -> [img, c, h, bw, wr] -> [img, c, h, w]
        y[:, c0:c0 + C_PER] = yk.transpose(3, 0, 2, 4, 1).reshape(
            IMGS, C_PER, H, W)
    y += bias_term[None, :, None, None]
    return y

